# revision 8
# baseline (speedup 1.0000x reference)
"""Trainium2 Bass kernel for nn_LocalAwareEncoder (GNN message passing).

Computes, for a sparse COO adjacency A (N x N, NNZ entries):
    h   = segment_sum(vals * ego[rows], cols)          # A^T @ ego
    h2  = segment_sum(vals * h[cols],  rows)           # A @ h
    out = LayerNorm(h2) * gamma + beta + ego
    returns (out[:N_USERS], out[N_USERS:])

Strategy (8 NeuronCores, SPMD):
  - Phase 1 sharded by destination (col) node range: each core owns N/8 dest
    nodes, processes only edges landing there.  Edges are host-sorted by dest
    and packed into 512-edge scatter tiles whose dest-runs never straddle tile
    boundaries.  Per 128-edge sub-tile the device gathers ego rows via
    indirect DMA, builds a one-hot (run-slot x val) selection matrix with a
    single tensor_scalar op against an iota tile, and matmuls into PSUM;
    4 sub-tiles accumulate, then one indirect-scatter DMA writes the per-dest
    sums (one row per dest, every owned dest covered exactly once).
  - AllGather replicates the per-core h slices to every core.
  - Phase 2 identical with (rows, cols) swapped, gathering from h_full.
  - LayerNorm + gamma/beta + ego residual fused over each core's slice.
"""

import sys

sys.path.insert(0, "/opt/trn_rl_repo")

import numpy as np

N_USERS = 100_000
N_ITEMS = 50_000
N = N_USERS + N_ITEMS
D = 128
EPS = 1e-5
C = 8                      # cores
NPC = N // C               # 18750 nodes per core
NPC_PAD = ((NPC + 127) // 128) * 128   # 18816
NBLK = NPC_PAD // 128      # 147 LayerNorm blocks per core
S = 8                      # 128-edge sub-tiles per scatter tile
SLOTS = 128 * S            # 1024 edge slots per scatter tile
RUNS = 128                 # max dest runs per scatter tile
OOB = 0x7FFFFFF0           # scatter index sentinel (skipped via bounds_check)

# Hardcoded tile counts for the reference dataset (seed 0).  Computed from the
# data at runtime; compile is cached per (T1, T2), rounded up for stability.
_compiled_cache = {}


def _pack_phase(dest, gidx, val):
    """Pack one core's edges (sorted by local dest) into scatter tiles.

    dest: local dest id per edge (ascending, in [0, NPC))
    gidx: gather index per edge (row id into the gather table)
    val:  edge value per edge
    Returns (rows_plane [T,SLOTS] i32, vals_plane [T,SLOTS] f32,
             runid_plane [T,SLOTS] f32, wix_plane [T,RUNS] i32)
    """
    counts = np.bincount(dest, minlength=NPC)
    cum = np.zeros(NPC + 1, dtype=np.int64)
    np.cumsum(counts, out=cum[1:])
    tiles = []
    d0 = 0
    while d0 < NPC:
        hi = int(np.searchsorted(cum, cum[d0] + SLOTS, side="right")) - 1
        d1 = min(hi, d0 + RUNS)
        if d1 <= d0:
            raise ValueError(f"dest {d0} degree {counts[d0]} exceeds {SLOTS}")
        tiles.append((d0, d1))
        d0 = d1
    T = len(tiles)
    rows_p = np.zeros((T, SLOTS), np.int32)
    vals_p = np.zeros((T, SLOTS), np.float32)
    runid_p = np.zeros((T, SLOTS), np.float32)
    wix_p = np.full((T, RUNS), OOB, np.int32)
    for t, (d0, d1) in enumerate(tiles):
        e0, e1 = int(cum[d0]), int(cum[d1])
        n = e1 - e0
        rows_p[t, :n] = gidx[e0:e1]
        vals_p[t, :n] = val[e0:e1]
        runid_p[t, :n] = (dest[e0:e1] - d0).astype(np.float32)
        wix_p[t, : d1 - d0] = np.arange(d0, d1, dtype=np.int32)
    return rows_p, vals_p, runid_p, wix_p


def _planes_for_cores(dest_node, gidx, val, order):
    """Shard edges by dest range and pack per core. Returns per-core plane
    lists and the max tile count."""
    ds = dest_node[order]
    gs = gidx[order]
    vs = val[order]
    # core boundaries in the dest-sorted edge array
    bounds = np.searchsorted(ds, np.arange(0, N + 1, NPC))
    packed = []
    for k in range(C):
        lo, hi = bounds[k], bounds[k + 1]
        packed.append(_pack_phase(ds[lo:hi] - k * NPC, gs[lo:hi], vs[lo:hi]))
    T = max(p[0].shape[0] for p in packed)
    return packed, T


def _pad_and_layout(packed, T):
    """Pad per-core planes to T tiles and convert to device layouts."""
    outs = []
    for rows_p, vals_p, runid_p, wix_p in packed:
        t0 = rows_p.shape[0]
        if t0 < T:
            pad = T - t0
            rows_p = np.concatenate([rows_p, np.zeros((pad, SLOTS), np.int32)])
            vals_p = np.concatenate([vals_p, np.zeros((pad, SLOTS), np.float32)])
            runid_p = np.concatenate([runid_p, np.zeros((pad, SLOTS), np.float32)])
            wix_p = np.concatenate([wix_p, np.full((pad, RUNS), OOB, np.int32)])
        # slot planes: (T, S, 128) -> [128, T*S] (partition = slot%128, col = t*S+s)
        rows_d = np.ascontiguousarray(
            rows_p.reshape(T, S, 128).transpose(2, 0, 1).reshape(128, T * S))
        vals_d = np.ascontiguousarray(
            vals_p.reshape(T, S, 128).transpose(2, 0, 1).reshape(128, T * S))
        runid_d = np.ascontiguousarray(
            runid_p.reshape(T, S, 128).transpose(2, 0, 1).reshape(128, T * S))
        wix_d = np.ascontiguousarray(wix_p.T)   # [128, T]
        outs.append((rows_d, vals_d, runid_d, wix_d))
    return outs


def _build_program(T1, T2):
    import concourse.bass as bass
    import concourse.bacc as bacc
    import concourse.mybir as mybir
    from concourse.tile import TileContext

    f32 = mybir.dt.float32
    i32 = mybir.dt.int32

    nc = bacc.Bacc("TRN2", target_bir_lowering=False, debug=False, num_devices=C)

    ego_d = nc.dram_tensor("ego", [N, D], f32, kind="ExternalInput")
    ego_sl_d = nc.dram_tensor("ego_slice", [NPC_PAD, D], f32, kind="ExternalInput")
    gamma_d = nc.dram_tensor("ln_gamma", [D], f32, kind="ExternalInput")
    beta_d = nc.dram_tensor("ln_beta", [D], f32, kind="ExternalInput")
    p1 = {nm: nc.dram_tensor(f"p1_{nm}", [128, T1 * S] if nm != "wix" else [128, T1],
                             i32 if nm in ("rows", "wix") else f32,
                             kind="ExternalInput")
          for nm in ("rows", "vals", "runid", "wix")}
    p2 = {nm: nc.dram_tensor(f"p2_{nm}", [128, T2 * S] if nm != "wix" else [128, T2],
                             i32 if nm in ("rows", "wix") else f32,
                             kind="ExternalInput")
          for nm in ("rows", "vals", "runid", "wix")}
    out_d = nc.dram_tensor("out_slice", [NPC_PAD, D], f32, kind="ExternalOutput")

    h_slice = nc.dram_tensor("h_slice", [NPC_PAD, D], f32)
    h_full = nc.dram_tensor("h_full", [C * NPC_PAD, D], f32, addr_space="Shared")
    h2_slice = nc.dram_tensor("h2_slice", [NPC_PAD, D], f32)

    def scatter_phase(tc, planes, T, table_ap, out_dram):
        """One TileContext worth of gather+matmul+scatter tiles."""
        with (
            tc.tile_pool(name="const", bufs=1) as constp,
            tc.tile_pool(name="planes", bufs=1) as planep,
            tc.tile_pool(name="gath", bufs=16) as gathp,
            tc.tile_pool(name="sel", bufs=16) as selp,
            tc.tile_pool(name="res", bufs=6) as resp,
            tc.tile_pool(name="psum", bufs=4, space="PSUM") as psp,
        ):
            iota = constp.tile([128, 128], f32)
            nc.gpsimd.iota(iota[:], pattern=[[1, 128]], base=0,
                           channel_multiplier=0,
                           allow_small_or_imprecise_dtypes=True)
            rows_sb = planep.tile([128, T * S], i32, tag="rows")
            vals_sb = planep.tile([128, T * S], f32, tag="vals")
            runid_sb = planep.tile([128, T * S], f32, tag="runid")
            wix_sb = planep.tile([128, T], i32, tag="wix")
            nc.sync.dma_start(out=rows_sb[:], in_=planes["rows"][:, :])
            nc.sync.dma_start(out=vals_sb[:], in_=planes["vals"][:, :])
            nc.sync.dma_start(out=runid_sb[:], in_=planes["runid"][:, :])
            nc.sync.dma_start(out=wix_sb[:], in_=planes["wix"][:, :])
            # Scatters are emitted LAG tiles behind their producers so the
            # in-order GpSimd sequencer never stalls waiting for the
            # matmul+copy chain of the tile it just gathered.
            LAG = 4
            pending = []

            def emit_scatter(res_tile, t):
                nc.gpsimd.indirect_dma_start(
                    out=out_dram[:, :],
                    out_offset=bass.IndirectOffsetOnAxis(
                        ap=wix_sb[:, t:t + 1], axis=0),
                    in_=res_tile[:], in_offset=None,
                    bounds_check=NPC_PAD - 1, oob_is_err=False,
                )

            for t in range(T):
                psum = psp.tile([128, 128], f32, tag="ps")
                for s in range(S):
                    col = t * S + s
                    g = gathp.tile([128, 128], f32, tag="g")
                    nc.gpsimd.indirect_dma_start(
                        out=g[:], out_offset=None,
                        in_=table_ap,
                        in_offset=bass.IndirectOffsetOnAxis(
                            ap=rows_sb[:, col:col + 1], axis=0),
                    )
                    sel = selp.tile([128, 128], f32, tag="sel")
                    nc.vector.tensor_scalar(
                        out=sel[:], in0=iota[:],
                        scalar1=runid_sb[:, col:col + 1],
                        scalar2=vals_sb[:, col:col + 1],
                        op0=mybir.AluOpType.is_equal,
                        op1=mybir.AluOpType.mult,
                    )
                    nc.tensor.matmul(out=psum[:], lhsT=sel[:], rhs=g[:],
                                     start=(s == 0), stop=(s == S - 1))
                res = resp.tile([128, 128], f32, tag="res")
                nc.vector.tensor_copy(out=res[:], in_=psum[:])
                pending.append((res, t))
                if len(pending) > LAG:
                    emit_scatter(*pending.pop(0))
            for res, t in pending:
                emit_scatter(res, t)

    # Each phase gets its own TileContext: context exit resets semaphores,
    # keeping 16-bit DMA sem wait values in range (~2000 SWDGE DMAs/phase).
    with TileContext(nc) as tc:
        scatter_phase(tc, p1, T1, ego_d[:, :], h_slice)

    with TileContext(nc) as tc:
        nc.gpsimd.collective_compute(
            "AllGather", mybir.AluOpType.bypass,
            replica_groups=[list(range(C))],
            ins=[h_slice.ap().opt()],
            outs=[h_full.ap().opt()],
        )

    with TileContext(nc) as tc:
        scatter_phase(tc, p2, T2, h_full[:, :], h2_slice)

    with TileContext(nc) as tc:
        with (
            tc.tile_pool(name="const", bufs=1) as constp,
            tc.tile_pool(name="ln", bufs=6) as lnp,
        ):
            # --- LayerNorm + gamma/beta + ego residual over this core's slice
            eps_t = constp.tile([128, 1], f32)
            nc.vector.memset(eps_t[:], EPS)
            gamma_b = constp.tile([128, D], f32)
            beta_b = constp.tile([128, D], f32)
            gamma_bcast = bass.AP(tensor=gamma_d, offset=0, ap=[[0, 128], [1, D]])
            beta_bcast = bass.AP(tensor=beta_d, offset=0, ap=[[0, 128], [1, D]])
            nc.gpsimd.dma_start(out=gamma_b[:], in_=gamma_bcast)
            nc.gpsimd.dma_start(out=beta_b[:], in_=beta_bcast)

            for b in range(NBLK):
                r0 = b * 128
                x = lnp.tile([128, D], f32, tag="x")
                nc.sync.dma_start(out=x[:], in_=h2_slice[r0:r0 + 128, :])
                ego_t = lnp.tile([128, D], f32, tag="egoT")
                nc.sync.dma_start(out=ego_t[:], in_=ego_sl_d[r0:r0 + 128, :])
                stats = lnp.tile([128, 6], f32, tag="st")
                nc.vector.bn_stats(out=stats[:], in_=x[:])
                mv = lnp.tile([128, 2], f32, tag="mv")
                nc.vector.bn_aggr(out=mv[:], in_=stats[:])
                # mv[:,0] = mean, mv[:,1] = var -> rstd
                nc.scalar.activation(out=mv[:, 1:2], in_=mv[:, 1:2],
                                     func=mybir.ActivationFunctionType.Sqrt,
                                     bias=eps_t[:], scale=1.0, alpha=0.0)
                nc.vector.reciprocal(out=mv[:, 1:2], in_=mv[:, 1:2])
                nc.vector.tensor_scalar(out=x[:], in0=x[:],
                                        scalar1=mv[:, 0:1], scalar2=mv[:, 1:2],
                                        op0=mybir.AluOpType.subtract,
                                        op1=mybir.AluOpType.mult)
                nc.vector.tensor_mul(out=x[:], in0=x[:], in1=gamma_b[:])
                nc.vector.tensor_add(out=x[:], in0=x[:], in1=beta_b[:])
                nc.vector.tensor_add(out=x[:], in0=x[:], in1=ego_t[:])
                nc.sync.dma_start(out=out_d[r0:r0 + 128, :], in_=x[:])

    nc.compile()
    return nc


TRACE = False          # set True (e.g. from test.py) to capture an NTFF trace
LAST_RESULTS = None    # BassKernelResults of the most recent run


def _ensure_axon_hooks_stub():
    """bass_utils imports antenv.axon_hooks when BASS_TRACE is set; this image
    lacks that module. Provide a no-op stub (only if missing) so tracing env
    vars degrade to no-trace instead of crashing."""
    try:
        import antenv.axon_hooks  # noqa: F401
    except ImportError:
        import types
        mod = types.ModuleType("antenv.axon_hooks")
        mod._hook = None
        mod.set_axon_ntff_profile_hook = lambda h: setattr(mod, "_hook", h)
        mod.get_axon_ntff_profile_hook = lambda: mod._hook
        sys.modules["antenv.axon_hooks"] = mod


def kernel(ego_embeddings, adj_vals, ln_gamma, ln_beta, adj_rows, adj_cols):
    _ensure_axon_hooks_stub()
    from concourse.bass_utils import run_bass_kernel_spmd

    ego = np.ascontiguousarray(np.asarray(ego_embeddings, dtype=np.float32))
    vals = np.asarray(adj_vals, dtype=np.float32)
    rows = np.asarray(adj_rows, dtype=np.int32)
    cols = np.asarray(adj_cols, dtype=np.int32)
    gamma = np.asarray(ln_gamma, dtype=np.float32)
    beta = np.asarray(ln_beta, dtype=np.float32)

    # ---- host-side sharding / packing
    # Phase 1: dest = col, gather table = ego indexed by raw row ids.
    order1 = np.argsort(cols, kind="stable")
    packed1, T1 = _planes_for_cores(cols, rows, vals, order1)
    # Phase 2: dest = row, gather table = h_full indexed by padded col ids.
    cols_padded = ((cols // NPC) * NPC_PAD + (cols % NPC)).astype(np.int32)
    order2 = np.argsort(rows, kind="stable")
    packed2, T2 = _planes_for_cores(rows, cols_padded, vals, order2)

    # round up for compile-cache stability
    T1c = ((T1 + 7) // 8) * 8
    T2c = ((T2 + 7) // 8) * 8
    layout1 = _pad_and_layout(packed1, T1c)
    layout2 = _pad_and_layout(packed2, T2c)

    key = (T1c, T2c)
    if key not in _compiled_cache:
        _compiled_cache[key] = _build_program(T1c, T2c)
    nc = _compiled_cache[key]

    in_maps = []
    for k in range(C):
        r1, v1, ri1, w1 = layout1[k]
        r2, v2, ri2, w2 = layout2[k]
        ego_slice = np.zeros((NPC_PAD, D), np.float32)
        ego_slice[:NPC] = ego[k * NPC:(k + 1) * NPC]
        in_maps.append({
            "ego": ego, "ego_slice": ego_slice,
            "ln_gamma": gamma, "ln_beta": beta,
            "p1_rows": r1, "p1_vals": v1, "p1_runid": ri1, "p1_wix": w1,
            "p2_rows": r2, "p2_vals": v2, "p2_runid": ri2, "p2_wix": w2,
        })

    global LAST_RESULTS
    res = run_bass_kernel_spmd(nc, in_maps, core_ids=list(range(C)),
                               trace=TRACE)
    LAST_RESULTS = res
    out = np.empty((N, D), np.float32)
    for k in range(C):
        out[k * NPC:(k + 1) * NPC] = res.results[k]["out_slice"][:NPC]
    return out[:N_USERS], out[N_USERS:]


# revision 10
# speedup vs baseline: 1.0331x; 1.0331x over previous
"""Trainium2 Bass kernel for nn_LocalAwareEncoder (GNN message passing).

Computes, for a sparse COO adjacency A (N x N, NNZ entries):
    h   = segment_sum(vals * ego[rows], cols)          # A^T @ ego
    h2  = segment_sum(vals * h[cols],  rows)           # A @ h
    out = LayerNorm(h2) * gamma + beta + ego
    returns (out[:N_USERS], out[N_USERS:])

Strategy (8 NeuronCores, SPMD):
  - Phase 1 sharded by destination (col) node range: each core owns N/8 dest
    nodes, processes only edges landing there.  Edges are host-sorted by dest
    and packed into 512-edge scatter tiles whose dest-runs never straddle tile
    boundaries.  Per 128-edge sub-tile the device gathers ego rows via
    indirect DMA, builds a one-hot (run-slot x val) selection matrix with a
    single tensor_scalar op against an iota tile, and matmuls into PSUM;
    4 sub-tiles accumulate, then one indirect-scatter DMA writes the per-dest
    sums (one row per dest, every owned dest covered exactly once).
  - AllGather replicates the per-core h slices to every core.
  - Phase 2 identical with (rows, cols) swapped, gathering from h_full.
  - LayerNorm + gamma/beta + ego residual fused over each core's slice.
"""

import sys

sys.path.insert(0, "/opt/trn_rl_repo")

import numpy as np

N_USERS = 100_000
N_ITEMS = 50_000
N = N_USERS + N_ITEMS
D = 128
EPS = 1e-5
C = 8                      # cores
NPC = N // C               # 18750 nodes per core
NPC_PAD = ((NPC + 127) // 128) * 128   # 18816
NBLK = NPC_PAD // 128      # 147 LayerNorm blocks per core
S = 8                      # 128-edge sub-tiles per scatter tile
SLOTS = 128 * S            # 1024 edge slots per scatter tile
RUNS = 128                 # max dest runs per scatter tile
OOB = 0x7FFFFFF0           # scatter index sentinel (skipped via bounds_check)

# Hardcoded tile counts for the reference dataset (seed 0).  Computed from the
# data at runtime; compile is cached per (T1, T2), rounded up for stability.
_compiled_cache = {}


def _pack_phase(dest, gidx, val):
    """Pack one core's edges (sorted by local dest) into scatter tiles.

    dest: local dest id per edge (ascending, in [0, NPC))
    gidx: gather index per edge (row id into the gather table)
    val:  edge value per edge
    Returns (rows_plane [T,SLOTS] i32, vals_plane [T,SLOTS] f32,
             runid_plane [T,SLOTS] f32, wix_plane [T,RUNS] i32)
    """
    counts = np.bincount(dest, minlength=NPC)
    cum = np.zeros(NPC + 1, dtype=np.int64)
    np.cumsum(counts, out=cum[1:])
    tiles = []
    d0 = 0
    while d0 < NPC:
        hi = int(np.searchsorted(cum, cum[d0] + SLOTS, side="right")) - 1
        d1 = min(hi, d0 + RUNS)
        if d1 <= d0:
            raise ValueError(f"dest {d0} degree {counts[d0]} exceeds {SLOTS}")
        tiles.append((d0, d1))
        d0 = d1
    T = len(tiles)
    rows_p = np.zeros((T, SLOTS), np.int32)
    vals_p = np.zeros((T, SLOTS), np.float32)
    runid_p = np.zeros((T, SLOTS), np.float32)
    wix_p = np.full((T, RUNS), OOB, np.int32)
    for t, (d0, d1) in enumerate(tiles):
        e0, e1 = int(cum[d0]), int(cum[d1])
        n = e1 - e0
        rows_p[t, :n] = gidx[e0:e1]
        vals_p[t, :n] = val[e0:e1]
        runid_p[t, :n] = (dest[e0:e1] - d0).astype(np.float32)
        wix_p[t, : d1 - d0] = np.arange(d0, d1, dtype=np.int32)
    return rows_p, vals_p, runid_p, wix_p


def _planes_for_cores(dest_node, gidx, val, order):
    """Shard edges by dest range and pack per core. Returns per-core plane
    lists and the max tile count."""
    ds = dest_node[order]
    gs = gidx[order]
    vs = val[order]
    # core boundaries in the dest-sorted edge array
    bounds = np.searchsorted(ds, np.arange(0, N + 1, NPC))
    packed = []
    for k in range(C):
        lo, hi = bounds[k], bounds[k + 1]
        packed.append(_pack_phase(ds[lo:hi] - k * NPC, gs[lo:hi], vs[lo:hi]))
    T = max(p[0].shape[0] for p in packed)
    return packed, T


def _pad_and_layout(packed, T):
    """Pad per-core planes to T tiles and convert to device layouts."""
    outs = []
    for rows_p, vals_p, runid_p, wix_p in packed:
        t0 = rows_p.shape[0]
        if t0 < T:
            pad = T - t0
            rows_p = np.concatenate([rows_p, np.zeros((pad, SLOTS), np.int32)])
            vals_p = np.concatenate([vals_p, np.zeros((pad, SLOTS), np.float32)])
            runid_p = np.concatenate([runid_p, np.zeros((pad, SLOTS), np.float32)])
            wix_p = np.concatenate([wix_p, np.full((pad, RUNS), OOB, np.int32)])
        # slot planes: (T, S, 128) -> [128, T*S] (partition = slot%128, col = t*S+s)
        rows_d = np.ascontiguousarray(
            rows_p.reshape(T, S, 128).transpose(2, 0, 1).reshape(128, T * S))
        vals_d = np.ascontiguousarray(
            vals_p.reshape(T, S, 128).transpose(2, 0, 1).reshape(128, T * S))
        runid_d = np.ascontiguousarray(
            runid_p.reshape(T, S, 128).transpose(2, 0, 1).reshape(128, T * S))
        wix_d = np.ascontiguousarray(wix_p.T)   # [128, T]
        outs.append((rows_d, vals_d, runid_d, wix_d))
    return outs


def _build_program(T1, T2):
    import concourse.bass as bass
    import concourse.bacc as bacc
    import concourse.mybir as mybir
    from concourse.tile import TileContext

    f32 = mybir.dt.float32
    i32 = mybir.dt.int32

    nc = bacc.Bacc("TRN2", target_bir_lowering=False, debug=False, num_devices=C)

    ego_d = nc.dram_tensor("ego", [N, D], f32, kind="ExternalInput")
    ego_sl_d = nc.dram_tensor("ego_slice", [NPC_PAD, D], f32, kind="ExternalInput")
    gamma_d = nc.dram_tensor("ln_gamma", [D], f32, kind="ExternalInput")
    beta_d = nc.dram_tensor("ln_beta", [D], f32, kind="ExternalInput")
    p1 = {nm: nc.dram_tensor(f"p1_{nm}", [128, T1 * S] if nm != "wix" else [128, T1],
                             i32 if nm in ("rows", "wix") else f32,
                             kind="ExternalInput")
          for nm in ("rows", "vals", "runid", "wix")}
    p2 = {nm: nc.dram_tensor(f"p2_{nm}", [128, T2 * S] if nm != "wix" else [128, T2],
                             i32 if nm in ("rows", "wix") else f32,
                             kind="ExternalInput")
          for nm in ("rows", "vals", "runid", "wix")}
    out_d = nc.dram_tensor("out_slice", [NPC_PAD, D], f32, kind="ExternalOutput")

    h_slice = nc.dram_tensor("h_slice", [NPC_PAD, D], f32)
    h_full = nc.dram_tensor("h_full", [C * NPC_PAD, D], f32, addr_space="Shared")
    h2_slice = nc.dram_tensor("h2_slice", [NPC_PAD, D], f32)

    def scatter_phase(tc, planes, T, table_ap, out_dram):
        """One TileContext worth of gather+matmul+scatter tiles."""
        with (
            tc.tile_pool(name="const", bufs=1) as constp,
            tc.tile_pool(name="planes", bufs=1) as planep,
            tc.tile_pool(name="gath", bufs=8) as gathp,
            tc.tile_pool(name="sel", bufs=8) as selp,
            tc.tile_pool(name="res", bufs=4) as resp,
            tc.tile_pool(name="psum", bufs=4, space="PSUM") as psp,
        ):
            iota = constp.tile([128, 128], f32)
            nc.gpsimd.iota(iota[:], pattern=[[1, 128]], base=0,
                           channel_multiplier=0,
                           allow_small_or_imprecise_dtypes=True)
            rows_sb = planep.tile([128, T * S], i32, tag="rows")
            vals_sb = planep.tile([128, T * S], f32, tag="vals")
            runid_sb = planep.tile([128, T * S], f32, tag="runid")
            wix_sb = planep.tile([128, T], i32, tag="wix")
            nc.sync.dma_start(out=rows_sb[:], in_=planes["rows"][:, :])
            nc.sync.dma_start(out=vals_sb[:], in_=planes["vals"][:, :])
            nc.sync.dma_start(out=runid_sb[:], in_=planes["runid"][:, :])
            nc.sync.dma_start(out=wix_sb[:], in_=planes["wix"][:, :])
            for t in range(T):
                psum = psp.tile([128, 128], f32, tag="ps")
                for s in range(S):
                    col = t * S + s
                    g = gathp.tile([128, 128], f32, tag="g")
                    nc.gpsimd.indirect_dma_start(
                        out=g[:], out_offset=None,
                        in_=table_ap,
                        in_offset=bass.IndirectOffsetOnAxis(
                            ap=rows_sb[:, col:col + 1], axis=0),
                    )
                    sel = selp.tile([128, 128], f32, tag="sel")
                    nc.vector.tensor_scalar(
                        out=sel[:], in0=iota[:],
                        scalar1=runid_sb[:, col:col + 1],
                        scalar2=vals_sb[:, col:col + 1],
                        op0=mybir.AluOpType.is_equal,
                        op1=mybir.AluOpType.mult,
                    )
                    nc.tensor.matmul(out=psum[:], lhsT=sel[:], rhs=g[:],
                                     start=(s == 0), stop=(s == S - 1))
                res = resp.tile([128, 128], f32, tag="res")
                nc.vector.tensor_copy(out=res[:], in_=psum[:])
                nc.gpsimd.indirect_dma_start(
                    out=out_dram[:, :],
                    out_offset=bass.IndirectOffsetOnAxis(
                        ap=wix_sb[:, t:t + 1], axis=0),
                    in_=res[:], in_offset=None,
                    bounds_check=NPC_PAD - 1, oob_is_err=False,
                )

    # Each phase gets its own TileContext: context exit resets semaphores,
    # keeping 16-bit DMA sem wait values in range (~2000 SWDGE DMAs/phase).
    with TileContext(nc) as tc:
        scatter_phase(tc, p1, T1, ego_d[:, :], h_slice)

    with TileContext(nc) as tc:
        nc.gpsimd.collective_compute(
            "AllGather", mybir.AluOpType.bypass,
            replica_groups=[list(range(C))],
            ins=[h_slice.ap().opt()],
            outs=[h_full.ap().opt()],
        )

    with TileContext(nc) as tc:
        scatter_phase(tc, p2, T2, h_full[:, :], h2_slice)

    with TileContext(nc) as tc:
        with (
            tc.tile_pool(name="const", bufs=1) as constp,
            tc.tile_pool(name="ln", bufs=6) as lnp,
        ):
            # --- LayerNorm + gamma/beta + ego residual over this core's slice
            eps_t = constp.tile([128, 1], f32)
            nc.vector.memset(eps_t[:], EPS)
            gamma_b = constp.tile([128, D], f32)
            beta_b = constp.tile([128, D], f32)
            gamma_bcast = bass.AP(tensor=gamma_d, offset=0, ap=[[0, 128], [1, D]])
            beta_bcast = bass.AP(tensor=beta_d, offset=0, ap=[[0, 128], [1, D]])
            nc.gpsimd.dma_start(out=gamma_b[:], in_=gamma_bcast)
            nc.gpsimd.dma_start(out=beta_b[:], in_=beta_bcast)

            for b in range(NBLK):
                r0 = b * 128
                x = lnp.tile([128, D], f32, tag="x")
                nc.sync.dma_start(out=x[:], in_=h2_slice[r0:r0 + 128, :])
                ego_t = lnp.tile([128, D], f32, tag="egoT")
                nc.sync.dma_start(out=ego_t[:], in_=ego_sl_d[r0:r0 + 128, :])
                stats = lnp.tile([128, 6], f32, tag="st")
                nc.vector.bn_stats(out=stats[:], in_=x[:])
                mv = lnp.tile([128, 2], f32, tag="mv")
                nc.vector.bn_aggr(out=mv[:], in_=stats[:])
                # mv[:,0] = mean, mv[:,1] = var -> rstd
                nc.scalar.activation(out=mv[:, 1:2], in_=mv[:, 1:2],
                                     func=mybir.ActivationFunctionType.Sqrt,
                                     bias=eps_t[:], scale=1.0, alpha=0.0)
                nc.vector.reciprocal(out=mv[:, 1:2], in_=mv[:, 1:2])
                nc.vector.tensor_scalar(out=x[:], in0=x[:],
                                        scalar1=mv[:, 0:1], scalar2=mv[:, 1:2],
                                        op0=mybir.AluOpType.subtract,
                                        op1=mybir.AluOpType.mult)
                nc.vector.tensor_mul(out=x[:], in0=x[:], in1=gamma_b[:])
                nc.vector.tensor_add(out=x[:], in0=x[:], in1=beta_b[:])
                nc.vector.tensor_add(out=x[:], in0=x[:], in1=ego_t[:])
                nc.sync.dma_start(out=out_d[r0:r0 + 128, :], in_=x[:])

    nc.compile()
    return nc


TRACE = False          # set True (e.g. from test.py) to capture an NTFF trace
LAST_RESULTS = None    # BassKernelResults of the most recent run


def _ensure_axon_hooks_stub():
    """bass_utils imports antenv.axon_hooks when BASS_TRACE is set; this image
    lacks that module. Provide a no-op stub (only if missing) so tracing env
    vars degrade to no-trace instead of crashing."""
    try:
        import antenv.axon_hooks  # noqa: F401
    except ImportError:
        import types
        mod = types.ModuleType("antenv.axon_hooks")
        mod._hook = None
        mod.set_axon_ntff_profile_hook = lambda h: setattr(mod, "_hook", h)
        mod.get_axon_ntff_profile_hook = lambda: mod._hook
        sys.modules["antenv.axon_hooks"] = mod


def kernel(ego_embeddings, adj_vals, ln_gamma, ln_beta, adj_rows, adj_cols):
    _ensure_axon_hooks_stub()
    from concourse.bass_utils import run_bass_kernel_spmd

    ego = np.ascontiguousarray(np.asarray(ego_embeddings, dtype=np.float32))
    vals = np.asarray(adj_vals, dtype=np.float32)
    rows = np.asarray(adj_rows, dtype=np.int32)
    cols = np.asarray(adj_cols, dtype=np.int32)
    gamma = np.asarray(ln_gamma, dtype=np.float32)
    beta = np.asarray(ln_beta, dtype=np.float32)

    # ---- host-side sharding / packing
    # Phase 1: dest = col, gather table = ego indexed by raw row ids.
    order1 = np.argsort(cols, kind="stable")
    packed1, T1 = _planes_for_cores(cols, rows, vals, order1)
    # Phase 2: dest = row, gather table = h_full indexed by padded col ids.
    cols_padded = ((cols // NPC) * NPC_PAD + (cols % NPC)).astype(np.int32)
    order2 = np.argsort(rows, kind="stable")
    packed2, T2 = _planes_for_cores(rows, cols_padded, vals, order2)

    # round up for compile-cache stability
    T1c = ((T1 + 7) // 8) * 8
    T2c = ((T2 + 7) // 8) * 8
    layout1 = _pad_and_layout(packed1, T1c)
    layout2 = _pad_and_layout(packed2, T2c)

    key = (T1c, T2c)
    if key not in _compiled_cache:
        _compiled_cache[key] = _build_program(T1c, T2c)
    nc = _compiled_cache[key]

    in_maps = []
    for k in range(C):
        r1, v1, ri1, w1 = layout1[k]
        r2, v2, ri2, w2 = layout2[k]
        ego_slice = np.zeros((NPC_PAD, D), np.float32)
        ego_slice[:NPC] = ego[k * NPC:(k + 1) * NPC]
        in_maps.append({
            "ego": ego, "ego_slice": ego_slice,
            "ln_gamma": gamma, "ln_beta": beta,
            "p1_rows": r1, "p1_vals": v1, "p1_runid": ri1, "p1_wix": w1,
            "p2_rows": r2, "p2_vals": v2, "p2_runid": ri2, "p2_wix": w2,
        })

    global LAST_RESULTS
    res = run_bass_kernel_spmd(nc, in_maps, core_ids=list(range(C)),
                               trace=TRACE)
    LAST_RESULTS = res
    out = np.empty((N, D), np.float32)
    for k in range(C):
        out[k * NPC:(k + 1) * NPC] = res.results[k]["out_slice"][:NPC]
    return out[:N_USERS], out[N_USERS:]
